# revision 1
# baseline (speedup 1.0000x reference)
"""CWTConvNet Trainium2 kernel.

The reference computes a 112-filter Morlet-wavelet SAME conv over length-2048
signals, then indexes the result with IMG_SELECT = linspace(0, 71, 224) cast
to int64 — i.e. only conv output positions 0..71 survive, each repeated 1-4
times. For those 72 positions only filter taps k in [209, 561) can touch
nonzero (non-pad) input, so the whole module reduces exactly to

    out72[f, s, l] = sum_{j=0}^{351} w2[f, j] * xe[s, j + l],   l in [0, 72)

with w2 = w_real[:, 0, 209:561] and xe = [71 zeros, x[s, 0:352], pad], then an
index-repeat expansion 72 -> 224 along the last axis.

Device kernel (per core, pure data parallel over 4 of 32 batches = 48
signals): im2col DMAs build the [128, 48*72] moving operand per 128-tap
contraction chunk, 21 accumulating matmuls produce PSUM [112, 3456], the
scalar engine drains PSUM to SBUF, the vector engine performs the monotone
run-length expansion to [112, 48, 224], and one DMA scatters straight into
the [4, 12, 112, 224] output layout.
"""

import numpy as np

import concourse.bacc as bacc
import concourse.bass as bass
import concourse.mybir as mybir
import concourse.tile as tile
from concourse.bass_utils import run_bass_kernel_spmd

# Problem constants (hardcoded; kernel.py must be self-contained).
B, C, L = 32, 12, 2048
F, K = 112, 561
NCORES = 8
BPC = B // NCORES          # batches per core
S = BPC * C                # signals per core (48)
NL = 72                    # conv output positions actually used
NI = 224                   # expanded output length
J = 352                    # taps that can touch non-pad input: k in [209, 561)
KOFF = 209                 # first needed tap
NCHUNK = 3                 # contraction chunks of 128 (352 -> 128,128,96)
XE_LEN = 456               # 71 zeros + 352 signal + tail zeros (>= 2*128+127+71+1)
XE_ZLEAD = 71
N_COLS = S * NL            # 3456 matmul moving columns per core
PSUM_TILE = 512            # fp32 columns per PSUM bank / matmul

SEL = np.linspace(0, 71, NI, dtype=np.int64)

_CACHE = {}


def _expansion_ops():
    """Decompose i -> SEL[i] into copy ops: ('seg3', t0, t1, i0) covers
    t in [t0,t1) each repeated 3x contiguously from dst column i0;
    ('rep', t, r, i0) covers one t repeated r times."""
    i0 = np.searchsorted(SEL, np.arange(NL), side="left")
    runs = np.bincount(SEL, minlength=NL)
    ops = []
    t = 0
    while t < NL:
        if runs[t] == 3:
            t1 = t
            while t1 < NL and runs[t1] == 3:
                t1 += 1
            ops.append(("seg3", t, t1, int(i0[t])))
            t = t1
        else:
            ops.append(("rep", t, int(runs[t]), int(i0[t])))
            t += 1
    return ops


def _build_nc():
    f32 = mybir.dt.float32
    nc = bacc.Bacc("TRN2", target_bir_lowering=False, debug=False)

    xe_d = nc.declare_dram_parameter("xe", [S, XE_LEN], f32, isOutput=False)
    w_d = nc.declare_dram_parameter("w2t", [128, NCHUNK, F], f32, isOutput=False)
    y_d = nc.declare_dram_parameter("y", [BPC, C, F, NI], f32, isOutput=True)

    with tile.TileContext(nc) as tc:
        with (
            tc.tile_pool(name="sbuf", bufs=1) as pool,
            tc.tile_pool(name="psum", bufs=1, space="PSUM") as psum_pool,
        ):
            w_t = pool.tile([128, NCHUNK, F], f32, tag="w")
            nc.sync.dma_start(out=w_t[:], in_=w_d.ap())

            # im2col: rhs[jc][p, s, l] = xe[s, 128*jc + p + l]
            rhs = []
            for jc in range(NCHUNK):
                r_t = pool.tile([128, S, NL], f32, tag=f"rhs{jc}")
                src = bass.AP(
                    tensor=xe_d,
                    offset=128 * jc,
                    ap=[[1, 128], [XE_LEN, S], [1, NL]],
                )
                nc.sync.dma_start(out=r_t[:], in_=src)
                rhs.append(r_t)

            psum_t = psum_pool.tile([128, N_COLS], f32, tag="acc")
            for jc in range(NCHUNK):
                r_flat = rhs[jc].rearrange("p s l -> p (s l)")
                for n0 in range(0, N_COLS, PSUM_TILE):
                    n1 = min(N_COLS, n0 + PSUM_TILE)
                    nc.tensor.matmul(
                        psum_t[:F, n0:n1],
                        w_t[:, jc, :],
                        r_flat[:, n0:n1],
                        start=(jc == 0),
                        stop=(jc == NCHUNK - 1),
                    )

            # Drain PSUM -> SBUF on the scalar engine (frees DVE for expansion).
            out72 = pool.tile([128, S, NL], f32, tag="out72")
            o72_flat = out72.rearrange("p s l -> p (s l)")
            for n0 in range(0, N_COLS, PSUM_TILE):
                n1 = min(N_COLS, n0 + PSUM_TILE)
                nc.scalar.copy(o72_flat[:F, n0:n1], psum_t[:F, n0:n1])

            # Expansion 72 -> 224 (vector engine, broadcast access patterns).
            out224 = pool.tile([128, S, NI], f32, tag="out224")
            for op in _expansion_ops():
                if op[0] == "seg3":
                    _, t0, t1, i0 = op
                    tlen = t1 - t0
                    dst = out224[:F, :, i0 : i0 + 3 * tlen].rearrange(
                        "p s (t d) -> p s t d", d=3
                    )
                    src = (
                        out72[:F, :, t0:t1]
                        .unsqueeze(3)
                        .broadcast_to([F, S, tlen, 3])
                    )
                else:
                    _, t, r, i0 = op
                    dst = out224[:F, :, i0 : i0 + r]
                    src = out72[:F, :, t : t + 1].broadcast_to([F, S, r])
                nc.vector.tensor_copy(out=dst, in_=src)

            # Store: [f, (b c), i] -> y[b, c, f, i]
            y_view = y_d.ap().rearrange("b c f i -> f (b c) i")
            nc.sync.dma_start(out=y_view, in_=out224[:F])

    nc.compile()
    return nc


def _get_nc():
    if "nc" not in _CACHE:
        _CACHE["nc"] = _build_nc()
    return _CACHE["nc"]


def _prepare_in_maps(x, w_real):
    x = np.ascontiguousarray(np.asarray(x), dtype=np.float32)
    w_real = np.asarray(w_real, dtype=np.float32)

    w2t = np.zeros((NCHUNK * 128, F), np.float32)
    w2t[:J] = w_real[:, 0, KOFF:K].T
    w2t_dev = np.ascontiguousarray(
        w2t.reshape(NCHUNK, 128, F).transpose(1, 0, 2)
    )

    in_maps = []
    for m in range(NCORES):
        xe = np.zeros((S, XE_LEN), np.float32)
        xe[:, XE_ZLEAD : XE_ZLEAD + J] = x[m * BPC : (m + 1) * BPC].reshape(
            S, L
        )[:, :J]
        in_maps.append({"xe": xe, "w2t": w2t_dev})
    return in_maps


def _assemble(results):
    return np.concatenate([r["y"] for r in results], axis=0)


def kernel(x, w_real):
    nc = _get_nc()
    in_maps = _prepare_in_maps(x, w_real)
    res = run_bass_kernel_spmd(nc, in_maps, list(range(NCORES)))
    return _assemble(res.results)


# revision 4
# speedup vs baseline: 1.2842x; 1.2842x over previous
"""CWTConvNet Trainium2 kernel.

The reference computes a 112-filter Morlet-wavelet SAME conv over length-2048
signals, then indexes the result with IMG_SELECT = linspace(0, 71, 224) cast
to int64 — i.e. only conv output positions 0..71 survive, each repeated 1-4
times. For those 72 positions only filter taps k in [209, 561) can touch
nonzero (non-pad) input, so the whole module reduces exactly to

    out72[f, s, l] = sum_{j=0}^{351} w2[f, j] * xe[s, j + l],   l in [0, 72)

with w2 = w_real[:, 0, 209:561] and xe = [71 zeros, x[s, 0:352], pad], then an
index-repeat expansion 72 -> 224 along the last axis.

Device kernel (per core, pure data parallel over 4 of 32 batches = 48
signals): im2col DMAs build the [128, n*72] moving operand per 128-tap
contraction chunk, accumulating matmuls produce PSUM, scalar/vector engines
drain PSUM to SBUF and perform the monotone run-length expansion to
[112, ., 224], and DMAs scatter straight into the [4, 12, 112, 224] output
layout. The 48 signals are processed as two pipelined halves; the two HWDGE
rings (sync + scalar) carry the DMAs in parallel.
"""

import numpy as np

import concourse.bacc as bacc
import concourse.bass as bass
import concourse.mybir as mybir
import concourse.tile as tile
from concourse.bass_utils import run_bass_kernel_spmd

# Problem constants (hardcoded; kernel.py must be self-contained).
B, C, L = 32, 12, 2048
F, K = 112, 561
NCORES = 8
BPC = B // NCORES          # batches per core
S = BPC * C                # signals per core (48)
NL = 72                    # conv output positions actually used
NI = 224                   # expanded output length
J = 352                    # taps that can touch non-pad input: k in [209, 561)
KOFF = 209                 # first needed tap
NCHUNK = 3                 # contraction chunks of 128 (352 -> 128,128,96)
XE_LEN = 456               # 71 zeros + 352 signal + tail zeros (>= 2*128+127+71+1)
XE_ZLEAD = 71

NHALF = 2                  # pipelined signal halves
SH = S // NHALF            # signals per half (24)
SPB = 6                    # signals per PSUM bank unit
UPH = SH // SPB            # units per half (4)
NCOL_U = SPB * NL          # matmul columns per unit (432)

# Config: input dtype for the matmul operands. fp32 is exact; bf16 halves
# im2col DMA bytes and matmul cycles at ~1e-3 relative error.
USE_BF16 = False

SEL = np.linspace(0, 71, NI, dtype=np.int64)

_CACHE = {}


def _expansion_ops():
    """Decompose i -> SEL[i] into copy ops: ('seg3', t0, t1, i0) covers
    t in [t0,t1) each repeated 3x contiguously from dst column i0;
    ('rep', t, r, i0) covers one t repeated r times."""
    i0 = np.searchsorted(SEL, np.arange(NL), side="left")
    runs = np.bincount(SEL, minlength=NL)
    ops = []
    t = 0
    while t < NL:
        if runs[t] == 3:
            t1 = t
            while t1 < NL and runs[t1] == 3:
                t1 += 1
            ops.append(("seg3", t, t1, int(i0[t])))
            t = t1
        else:
            ops.append(("rep", t, int(runs[t]), int(i0[t])))
            t += 1
    return ops


def _build_nc():
    f32 = mybir.dt.float32
    dt_in = mybir.dt.bfloat16 if USE_BF16 else f32
    nc = bacc.Bacc("TRN2", target_bir_lowering=False, debug=False)

    xe_d = nc.declare_dram_parameter("xe", [S, XE_LEN], dt_in, isOutput=False)
    w_d = nc.declare_dram_parameter("w2t", [128, NCHUNK, F], dt_in, isOutput=False)
    y_d = nc.declare_dram_parameter("y", [BPC, C, F, NI], f32, isOutput=True)

    exp_ops = _expansion_ops()

    with tile.TileContext(nc) as tc:
        with (
            tc.tile_pool(name="sbuf", bufs=1) as pool,
            tc.tile_pool(name="psum", bufs=1, space="PSUM") as psum_pool,
        ):
            w_t = pool.tile([128, NCHUNK, F], dt_in, tag="w", name="w")
            nc.sync.dma_start(out=w_t[:], in_=w_d.ap())

            # im2col per (half, chunk): rhs[p, s, l] = xe[24h + s, 128*jc + p + l]
            rhs = {}
            for h in range(NHALF):
                for jc in range(NCHUNK):
                    r_t = pool.tile([128, SH, NL], dt_in, tag=f"rhs{h}_{jc}", name=f"rhs{h}_{jc}")
                    src = bass.AP(
                        tensor=xe_d,
                        offset=128 * jc + (h * SH) * XE_LEN,
                        ap=[[1, 128], [XE_LEN, SH], [1, NL]],
                    )
                    eng = nc.sync if (h + jc) % 2 == 0 else nc.scalar
                    eng.dma_start(out=r_t[:], in_=src)
                    rhs[(h, jc)] = r_t

            psum_u = [
                psum_pool.tile([128, NCOL_U], f32, tag=f"ps{u}", name=f"ps{u}")
                for u in range(NHALF * UPH)
            ]

            out72 = {}
            out224 = {}
            for h in range(NHALF):
                # Matmuls: units of 6 signals, one PSUM bank each.
                for jc in range(NCHUNK):
                    r_flat = rhs[(h, jc)].rearrange("p s l -> p (s l)")
                    for uu in range(UPH):
                        u = h * UPH + uu
                        n0 = uu * NCOL_U
                        nc.tensor.matmul(
                            psum_u[u][:F, :],
                            w_t[:, jc, :],
                            r_flat[:, n0 : n0 + NCOL_U],
                            start=(jc == 0),
                            stop=(jc == NCHUNK - 1),
                        )

                # Drain PSUM -> SBUF, alternating scalar/vector by unit.
                o72 = pool.tile([128, SH, NL], f32, tag=f"out72_{h}", name=f"out72_{h}")
                o72_flat = o72.rearrange("p s l -> p (s l)")
                for uu in range(UPH):
                    u = h * UPH + uu
                    n0 = uu * NCOL_U
                    eng = nc.scalar if uu % 2 == 0 else nc.vector
                    if uu % 2 == 0:
                        nc.scalar.copy(
                            o72_flat[:F, n0 : n0 + NCOL_U], psum_u[u][:F, :]
                        )
                    else:
                        nc.vector.tensor_copy(
                            out=o72_flat[:F, n0 : n0 + NCOL_U], in_=psum_u[u][:F, :]
                        )
                out72[h] = o72

                # Expansion 72 -> 224 via broadcast copies, vector/scalar split.
                o224 = pool.tile([128, SH, NI], f32, tag=f"out224_{h}", name=f"out224_{h}")
                out224[h] = o224
                for k, op in enumerate(exp_ops):
                    if op[0] == "seg3":
                        _, t0, t1, i0 = op
                        tlen = t1 - t0
                        dst = o224[:F, :, i0 : i0 + 3 * tlen].rearrange(
                            "p s (t d) -> p s t d", d=3
                        )
                        src = (
                            o72[:F, :, t0:t1]
                            .unsqueeze(3)
                            .broadcast_to([F, SH, tlen, 3])
                        )
                    else:
                        _, t, r, i0 = op
                        dst = o224[:F, :, i0 : i0 + r]
                        src = o72[:F, :, t : t + 1].broadcast_to([F, SH, r])
                    if op[0] == "seg3":
                        nc.vector.tensor_copy(out=dst, in_=src)
                    else:
                        nc.scalar.copy(dst, src)

                # Store: [f, s, i] -> y[b, c, f, i] for this half's signals.
                y_view = y_d.ap().rearrange("b c f i -> f (b c) i")[
                    :, h * SH : (h + 1) * SH, :
                ]
                eng = nc.sync if h % 2 == 0 else nc.scalar
                eng.dma_start(out=y_view, in_=o224[:F])

    nc.compile()
    return nc


def _get_nc():
    if "nc" not in _CACHE:
        _CACHE["nc"] = _build_nc()
    return _CACHE["nc"]


def _prepare_in_maps(x, w_real):
    if USE_BF16:
        import ml_dtypes

        np_in = np.dtype(ml_dtypes.bfloat16)
    else:
        np_in = np.dtype(np.float32)
    x = np.ascontiguousarray(np.asarray(x), dtype=np.float32)
    w_real = np.asarray(w_real, dtype=np.float32)

    w2t = np.zeros((NCHUNK * 128, F), np.float32)
    w2t[:J] = w_real[:, 0, KOFF:K].T
    w2t_dev = np.ascontiguousarray(
        w2t.reshape(NCHUNK, 128, F).transpose(1, 0, 2)
    ).astype(np_in)

    in_maps = []
    for m in range(NCORES):
        xe = np.zeros((S, XE_LEN), np.float32)
        xe[:, XE_ZLEAD : XE_ZLEAD + J] = x[m * BPC : (m + 1) * BPC].reshape(
            S, L
        )[:, :J]
        in_maps.append({"xe": xe.astype(np_in), "w2t": w2t_dev})
    return in_maps


def _assemble(results):
    return np.concatenate([r["y"] for r in results], axis=0)


def kernel(x, w_real):
    nc = _get_nc()
    in_maps = _prepare_in_maps(x, w_real)
    res = run_bass_kernel_spmd(nc, in_maps, list(range(NCORES)))
    return _assemble(res.results)


# revision 6
# speedup vs baseline: 1.3896x; 1.0820x over previous
"""CWTConvNet Trainium2 kernel.

The reference computes a 112-filter Morlet-wavelet SAME conv over length-2048
signals, then indexes the result with IMG_SELECT = linspace(0, 71, 224) cast
to int64 — i.e. only conv output positions 0..71 survive, each repeated 1-4
times. For those 72 positions only filter taps k in [209, 561) can touch
nonzero (non-pad) input, so the whole module reduces exactly to

    out72[f, s, l] = sum_{j=0}^{351} w2[f, j] * xe[s, j + l],   l in [0, 72)

with w2 = w_real[:, 0, 209:561] and xe = [71 zeros, x[s, 0:352], pad], then an
index-repeat expansion 72 -> 224 along the last axis.

Device kernel (per core, pure data parallel over 4 of 32 batches = 48
signals): im2col DMAs build the [128, n*72] moving operand per 128-tap
contraction chunk, accumulating matmuls produce PSUM, scalar/vector engines
drain PSUM to SBUF and perform the monotone run-length expansion to
[112, ., 224], and DMAs scatter straight into the [4, 12, 112, 224] output
layout. The 48 signals are processed as two pipelined halves; the two HWDGE
rings (sync + scalar) carry the DMAs in parallel.
"""

import numpy as np

import concourse.bacc as bacc
import concourse.bass as bass
import concourse.mybir as mybir
import concourse.tile as tile
from concourse.bass_utils import run_bass_kernel_spmd

# Problem constants (hardcoded; kernel.py must be self-contained).
B, C, L = 32, 12, 2048
F, K = 112, 561
NCORES = 8
BPC = B // NCORES          # batches per core
S = BPC * C                # signals per core (48)
NL = 72                    # conv output positions actually used
NI = 224                   # expanded output length
J = 352                    # taps that can touch non-pad input: k in [209, 561)
KOFF = 209                 # first needed tap
NCHUNK = 3                 # contraction chunks of 128 (352 -> 128,128,96)
XE_LEN = 456               # 71 zeros + 352 signal + tail zeros (>= 2*128+127+71+1)
XE_ZLEAD = 71

NHALF = 2                  # pipelined signal halves
SH = S // NHALF            # signals per half (24)
SPB = 6                    # signals per PSUM bank unit
UPH = SH // SPB            # units per half (4)
NCOL_U = SPB * NL          # matmul columns per unit (432)

# Config: input dtype for the matmul operands. fp32 is exact; bf16 halves
# im2col DMA bytes and matmul cycles at ~1e-3 relative error.
USE_BF16 = True

SEL = np.linspace(0, 71, NI, dtype=np.int64)

_CACHE = {}


def _expansion_ops():
    """Decompose i -> SEL[i] into copy ops: ('seg3', t0, t1, i0) covers
    t in [t0,t1) each repeated 3x contiguously from dst column i0;
    ('rep', t, r, i0) covers one t repeated r times."""
    i0 = np.searchsorted(SEL, np.arange(NL), side="left")
    runs = np.bincount(SEL, minlength=NL)
    ops = []
    t = 0
    while t < NL:
        if runs[t] == 3:
            t1 = t
            while t1 < NL and runs[t1] == 3:
                t1 += 1
            ops.append(("seg3", t, t1, int(i0[t])))
            t = t1
        else:
            ops.append(("rep", t, int(runs[t]), int(i0[t])))
            t += 1
    return ops


def _build_nc():
    f32 = mybir.dt.float32
    dt_in = mybir.dt.bfloat16 if USE_BF16 else f32
    nc = bacc.Bacc("TRN2", target_bir_lowering=False, debug=False)

    xe_d = nc.declare_dram_parameter("xe", [S, XE_LEN], dt_in, isOutput=False)
    w_d = nc.declare_dram_parameter("w2t", [128, NCHUNK, F], dt_in, isOutput=False)
    y_d = nc.declare_dram_parameter("y", [BPC, C, F, NI], f32, isOutput=True)

    exp_ops = _expansion_ops()

    with tile.TileContext(nc) as tc:
        with (
            tc.tile_pool(name="sbuf", bufs=1) as pool,
            tc.tile_pool(name="psum", bufs=1, space="PSUM") as psum_pool,
        ):
            w_t = pool.tile([128, NCHUNK, F], dt_in, tag="w", name="w")
            nc.sync.dma_start(out=w_t[:], in_=w_d.ap())

            # im2col per (half, chunk): rhs[p, s, l] = xe[24h + s, 128*jc + p + l]
            rhs = {}
            for h in range(NHALF):
                for jc in range(NCHUNK):
                    r_t = pool.tile([128, SH, NL], dt_in, tag=f"rhs{h}_{jc}", name=f"rhs{h}_{jc}")
                    src = bass.AP(
                        tensor=xe_d,
                        offset=128 * jc + (h * SH) * XE_LEN,
                        ap=[[1, 128], [XE_LEN, SH], [1, NL]],
                    )
                    eng = nc.sync if (h + jc) % 2 == 0 else nc.scalar
                    eng.dma_start(out=r_t[:], in_=src)
                    rhs[(h, jc)] = r_t

            psum_u = [
                psum_pool.tile([128, NCOL_U], f32, tag=f"ps{u}", name=f"ps{u}")
                for u in range(NHALF * UPH)
            ]

            out72 = {}
            out224 = {}
            for h in range(NHALF):
                # Matmuls: units of 6 signals, one PSUM bank each.
                for jc in range(NCHUNK):
                    r_flat = rhs[(h, jc)].rearrange("p s l -> p (s l)")
                    for uu in range(UPH):
                        u = h * UPH + uu
                        n0 = uu * NCOL_U
                        nc.tensor.matmul(
                            psum_u[u][:F, :],
                            w_t[:, jc, :],
                            r_flat[:, n0 : n0 + NCOL_U],
                            start=(jc == 0),
                            stop=(jc == NCHUNK - 1),
                        )

                # Drain PSUM -> SBUF, alternating scalar/vector by unit.
                o72 = pool.tile([128, SH, NL], f32, tag=f"out72_{h}", name=f"out72_{h}")
                o72_flat = o72.rearrange("p s l -> p (s l)")
                for uu in range(UPH):
                    u = h * UPH + uu
                    n0 = uu * NCOL_U
                    eng = nc.scalar if uu % 2 == 0 else nc.vector
                    if uu % 2 == 0:
                        nc.scalar.copy(
                            o72_flat[:F, n0 : n0 + NCOL_U], psum_u[u][:F, :]
                        )
                    else:
                        nc.vector.tensor_copy(
                            out=o72_flat[:F, n0 : n0 + NCOL_U], in_=psum_u[u][:F, :]
                        )
                out72[h] = o72

                # Expansion 72 -> 224 via broadcast copies, vector/scalar split.
                o224 = pool.tile([128, SH, NI], f32, tag=f"out224_{h}", name=f"out224_{h}")
                out224[h] = o224
                for k, op in enumerate(exp_ops):
                    if op[0] == "seg3":
                        _, t0, t1, i0 = op
                        tlen = t1 - t0
                        dst = o224[:F, :, i0 : i0 + 3 * tlen].rearrange(
                            "p s (t d) -> p s t d", d=3
                        )
                        src = (
                            o72[:F, :, t0:t1]
                            .unsqueeze(3)
                            .broadcast_to([F, SH, tlen, 3])
                        )
                    else:
                        _, t, r, i0 = op
                        dst = o224[:F, :, i0 : i0 + r]
                        src = o72[:F, :, t : t + 1].broadcast_to([F, SH, r])
                    if op[0] == "seg3":
                        nc.vector.tensor_copy(out=dst, in_=src)
                    else:
                        nc.scalar.copy(dst, src)

                # Store: [f, s, i] -> y[b, c, f, i], split across both HWDGE
                # rings so the tail store is halved.
                y_all = y_d.ap().rearrange("b c f i -> f (b c) i")
                sq = SH // 2
                for q in range(2):
                    s0 = h * SH + q * sq
                    eng = nc.sync if q == 0 else nc.scalar
                    eng.dma_start(
                        out=y_all[:, s0 : s0 + sq, :],
                        in_=o224[:F, q * sq : (q + 1) * sq, :],
                    )

    nc.compile()
    return nc


def _get_nc():
    if "nc" not in _CACHE:
        _CACHE["nc"] = _build_nc()
    return _CACHE["nc"]


def _prepare_in_maps(x, w_real):
    if USE_BF16:
        import ml_dtypes

        np_in = np.dtype(ml_dtypes.bfloat16)
    else:
        np_in = np.dtype(np.float32)
    x = np.ascontiguousarray(np.asarray(x), dtype=np.float32)
    w_real = np.asarray(w_real, dtype=np.float32)

    w2t = np.zeros((NCHUNK * 128, F), np.float32)
    w2t[:J] = w_real[:, 0, KOFF:K].T
    w2t_dev = np.ascontiguousarray(
        w2t.reshape(NCHUNK, 128, F).transpose(1, 0, 2)
    ).astype(np_in)

    in_maps = []
    for m in range(NCORES):
        xe = np.zeros((S, XE_LEN), np.float32)
        xe[:, XE_ZLEAD : XE_ZLEAD + J] = x[m * BPC : (m + 1) * BPC].reshape(
            S, L
        )[:, :J]
        in_maps.append({"xe": xe.astype(np_in), "w2t": w2t_dev})
    return in_maps


def _assemble(results):
    return np.concatenate([r["y"] for r in results], axis=0)


def kernel(x, w_real):
    nc = _get_nc()
    in_maps = _prepare_in_maps(x, w_real)
    res = run_bass_kernel_spmd(nc, in_maps, list(range(NCORES)))
    return _assemble(res.results)


# revision 8
# speedup vs baseline: 1.5850x; 1.1406x over previous
"""CWTConvNet Trainium2 kernel.

The reference computes a 112-filter Morlet-wavelet SAME conv over length-2048
signals, then indexes the result with IMG_SELECT = linspace(0, 71, 224) cast
to int64 — i.e. only conv output positions 0..71 survive, each repeated 1-4
times. For those 72 positions only filter taps k in [209, 561) can touch
nonzero (non-pad) input, so the whole module reduces exactly to

    out72[f, s, l] = sum_{j=0}^{351} w2[f, j] * xe[s, j + l],   l in [0, 72)

with w2 = w_real[:, 0, 209:561] and xe = [71 zeros, x[s, 0:352], pad], then an
index-repeat expansion 72 -> 224 along the last axis.

Device kernel (per core, pure data parallel over 4 of 32 batches = 48
signals): the host supplies xe with groups of 6 signals interleaved
element-wise, so each im2col DMA descriptor carries 6 signals (864B contiguous
runs instead of 144B — the im2col is descriptor-rate-bound otherwise).
Per 128-tap contraction chunk a DMA builds the [128, 4*432] moving operand;
accumulating matmuls produce one PSUM bank per 6-signal group; scalar/vector
engines drain + de-interleave PSUM to SBUF and perform the monotone
run-length expansion to [112, ., 224]; DMAs scatter straight into the
[4, 12, 112, 224] output layout. Work is pipelined over two signal halves
(DMA granularity) and four quarters (store granularity) across both HWDGE
rings; dummy matmuls on a zeroed tile warm the PE clock gate during the
initial DMA wait.
"""

import numpy as np

import concourse.bacc as bacc
import concourse.bass as bass
import concourse.mybir as mybir
import concourse.tile as tile
from concourse.bass_utils import run_bass_kernel_spmd

# Problem constants (hardcoded; kernel.py must be self-contained).
B, C, L = 32, 12, 2048
F, K = 112, 561
NCORES = 8
BPC = B // NCORES          # batches per core
S = BPC * C                # signals per core (48)
NL = 72                    # conv output positions actually used
NI = 224                   # expanded output length
J = 352                    # taps that can touch non-pad input: k in [209, 561)
KOFF = 209                 # first needed tap
NCHUNK = 3                 # contraction chunks of 128 (352 -> 128,128,96)
XE_LEN = 456               # 71 zeros + 352 signal + tail zeros (>= 2*128+127+71+1)
XE_ZLEAD = 71

TI = 6                     # signals interleaved per im2col descriptor
NG = S // TI               # signal groups per core (8)
NHALF = 2                  # DMA-granularity halves
GPH = NG // NHALF          # groups per half (4)
NCOL_U = TI * NL           # matmul columns per group/PSUM bank (432)
NQ = 4                     # store-granularity quarters (2 groups each)

# Config: input dtype for the matmul operands. fp32 is exact; bf16 halves
# im2col DMA bytes and matmul passes at ~2e-3 relative error.
USE_BF16 = True
WARMUP_MM = 10             # dummy matmuls to lift the PE HAM clock gate

SEL = np.linspace(0, 71, NI, dtype=np.int64)

_CACHE = {}


def _expansion_ops():
    """Decompose i -> SEL[i] into copy ops: ('seg3', t0, t1, i0) covers
    t in [t0,t1) each repeated 3x contiguously from dst column i0;
    ('rep', t, r, i0) covers one t repeated r times."""
    i0 = np.searchsorted(SEL, np.arange(NL), side="left")
    runs = np.bincount(SEL, minlength=NL)
    ops = []
    t = 0
    while t < NL:
        if runs[t] == 3:
            t1 = t
            while t1 < NL and runs[t1] == 3:
                t1 += 1
            ops.append(("seg3", t, t1, int(i0[t])))
            t = t1
        else:
            ops.append(("rep", t, int(runs[t]), int(i0[t])))
            t += 1
    return ops


def _build_nc():
    f32 = mybir.dt.float32
    dt_in = mybir.dt.bfloat16 if USE_BF16 else f32
    nc = bacc.Bacc("TRN2", target_bir_lowering=False, debug=False)

    # xg[g, t, k] = xe[6g + k, t]  (6-signal element interleave), flat [NG, 2736]
    xg_d = nc.declare_dram_parameter("xg", [NG, XE_LEN * TI], dt_in, isOutput=False)
    w_d = nc.declare_dram_parameter("w2t", [128, NCHUNK, F], dt_in, isOutput=False)
    y_d = nc.declare_dram_parameter("y", [BPC, C, F, NI], f32, isOutput=True)

    exp_ops = _expansion_ops()

    with tile.TileContext(nc) as tc:
        with (
            tc.tile_pool(name="sbuf", bufs=1) as pool,
            tc.tile_pool(name="psum", bufs=1, space="PSUM") as psum_pool,
        ):
            w_t = pool.tile([128, NCHUNK, F], dt_in, tag="w", name="w")
            nc.sync.dma_start(out=w_t[:], in_=w_d.ap())

            psum_u = [
                psum_pool.tile([128, NCOL_U], f32, tag=f"ps{u}", name=f"ps{u}")
                for u in range(NG)
            ]

            # PE warm-up: matmuls on a zeroed tile lift the HAM clock gate
            # (1.2 -> 2.4 GHz) while the im2col DMAs are in flight.
            if WARMUP_MM:
                scratch = pool.tile([128, 512], dt_in, tag="warm", name="warm")
                nc.gpsimd.memset(scratch[:], 0.0)
                for _ in range(WARMUP_MM):
                    nc.tensor.matmul(
                        psum_u[0][:F, :],
                        scratch[:, :F],
                        scratch[:, :NCOL_U],
                        start=True,
                        stop=True,
                    )

            # im2col per (half, chunk): rhs[p, g, (l k)] = xg[4h+g, (128jc+p+l)*6 + k]
            rhs = {}
            ring = {0: nc.sync, 1: nc.scalar}
            for h in range(NHALF):
                for jc in range(NCHUNK):
                    r_t = pool.tile(
                        [128, GPH, NCOL_U], dt_in,
                        tag=f"rhs{h}_{jc}", name=f"rhs{h}_{jc}",
                    )
                    src = bass.AP(
                        tensor=xg_d,
                        offset=h * GPH * XE_LEN * TI + 128 * jc * TI,
                        ap=[[TI, 128], [XE_LEN * TI, GPH], [1, NCOL_U]],
                    )
                    ring[(h + jc) % 2].dma_start(out=r_t[:], in_=src)
                    rhs[(h, jc)] = r_t

            y_all = y_d.ap().rearrange("b c f i -> f (b c) i")
            for h in range(NHALF):
                o72 = pool.tile([128, S // NHALF, NL], f32,
                                tag=f"out72_{h}", name=f"out72_{h}")
                o224 = pool.tile([128, S // NHALF, NI], f32,
                                 tag=f"out224_{h}", name=f"out224_{h}")
                for qq in range(NQ // NHALF):
                    units = [2 * qq, 2 * qq + 1]  # groups within this half
                    # Matmuls: one PSUM bank per 6-signal group.
                    for jc in range(NCHUNK):
                        r_flat = rhs[(h, jc)].rearrange("p g n -> p (g n)")
                        for uu in units:
                            u = h * GPH + uu
                            nc.tensor.matmul(
                                psum_u[u][:F, :],
                                w_t[:, jc, :],
                                r_flat[:, uu * NCOL_U : (uu + 1) * NCOL_U],
                                start=(jc == 0),
                                stop=(jc == NCHUNK - 1),
                            )
                    # Drain + de-interleave PSUM -> SBUF: cols (l, k) -> [s, l].
                    for uu in units:
                        u = h * GPH + uu
                        src = psum_u[u][:F].rearrange("p (l k) -> p l k", k=TI)
                        dst = o72[:F, uu * TI : (uu + 1) * TI, :].rearrange(
                            "p k l -> p l k"
                        )
                        if uu % 2 == 0:
                            nc.scalar.copy(dst, src)
                        else:
                            nc.vector.tensor_copy(out=dst, in_=src)

                    # Expansion 72 -> 224 for this quarter (12 signals).
                    s0 = qq * 2 * TI
                    o72q = o72[:F, s0 : s0 + 2 * TI, :]
                    o224q = o224[:F, s0 : s0 + 2 * TI, :]
                    for op in exp_ops:
                        if op[0] == "seg3":
                            _, t0, t1, i0 = op
                            tlen = t1 - t0
                            dst = o224q[:, :, i0 : i0 + 3 * tlen].rearrange(
                                "p s (t d) -> p s t d", d=3
                            )
                            src = (
                                o72q[:, :, t0:t1]
                                .unsqueeze(3)
                                .broadcast_to([F, 2 * TI, tlen, 3])
                            )
                            nc.vector.tensor_copy(out=dst, in_=src)
                        else:
                            _, t, r, i0 = op
                            dst = o224q[:, :, i0 : i0 + r]
                            src = o72q[:, :, t : t + 1].broadcast_to([F, 2 * TI, r])
                            nc.scalar.copy(dst, src)

                    # Store this quarter: [f, s, i] -> y[b, c, f, i].
                    g0 = h * (S // NHALF) + s0
                    ring[(h * 2 + qq) % 2].dma_start(
                        out=y_all[:, g0 : g0 + 2 * TI, :],
                        in_=o224[:F, s0 : s0 + 2 * TI, :],
                    )

    nc.compile()
    return nc


def _get_nc():
    if "nc" not in _CACHE:
        _CACHE["nc"] = _build_nc()
    return _CACHE["nc"]


def _prepare_in_maps(x, w_real):
    if USE_BF16:
        import ml_dtypes

        np_in = np.dtype(ml_dtypes.bfloat16)
    else:
        np_in = np.dtype(np.float32)
    x = np.ascontiguousarray(np.asarray(x), dtype=np.float32)
    w_real = np.asarray(w_real, dtype=np.float32)

    w2t = np.zeros((NCHUNK * 128, F), np.float32)
    w2t[:J] = w_real[:, 0, KOFF:K].T
    w2t_dev = np.ascontiguousarray(
        w2t.reshape(NCHUNK, 128, F).transpose(1, 0, 2)
    ).astype(np_in)

    in_maps = []
    for m in range(NCORES):
        xe = np.zeros((S, XE_LEN), np.float32)
        xe[:, XE_ZLEAD : XE_ZLEAD + J] = x[m * BPC : (m + 1) * BPC].reshape(
            S, L
        )[:, :J]
        # interleave: xg[g, t, k] = xe[6g + k, t]
        xg = np.ascontiguousarray(
            xe.reshape(NG, TI, XE_LEN).transpose(0, 2, 1)
        ).reshape(NG, XE_LEN * TI)
        in_maps.append({"xg": xg.astype(np_in), "w2t": w2t_dev})
    return in_maps


def _assemble(results):
    return np.concatenate([r["y"] for r in results], axis=0)


def kernel(x, w_real):
    nc = _get_nc()
    in_maps = _prepare_in_maps(x, w_real)
    res = run_bass_kernel_spmd(nc, in_maps, list(range(NCORES)))
    return _assemble(res.results)
